# revision 13
# baseline (speedup 1.0000x reference)
"""MoE routing kernel for Trainium2 (8 NeuronCores) — host-routed,
set-sorted, run-sparse expert GEMM.

Reference computation:
    scores = x @ gate_w.T                    [N, E]
    top-4 experts per token; routing weight = top-1 score for ALL selected
    hs = sum_{e in top4} (x @ expert_w[e].T) * top1
    out = relu(hs)^2 @ out_w.T

Key idea: top-4-of-8 routing means only half the expert FLOPs are needed.
The host (inside kernel(), numpy) computes the gating exactly (fp64 scores,
top-4, top-1 weight), groups tokens by their 4-expert set (70 distinct
sets), and lays tokens out so each set occupies a contiguous window of
columns. On device, expert e's GEMM runs only over the merged column runs
of windows whose set contains e — no masks, no gate, no per-expert operand
copies; the moving operand is xT16 itself. Sum over a token's 4 experts
happens in PSUM (bank pre-zeroed by one wide zero-weight matmul, so run
matmuls never need start flags).

SPMD: all 8 cores run one program, so window widths must match across
cores. Each set's token count n_s splits as 8*base_s + r_s; every core's
window is base_s + (r_s>0) slots, with cores lacking a remainder token
holding a zero (dummy) column: x col = 0, t1sq = 0, output row discarded.

The top-1 weight splits exactly (as in the dense baseline): sign(top1)
is folded into xT16 on host (exact in bf16), top1^2 is applied fp32
per-partition at the out-projection eviction.

Device pipeline per core: load xT16 [P, DD, T'] bf16 + t1sq + out_w; for
each of 16 i-tiles: stream expert_w tile (2MB, loaded ONCE), zero-matmul
each PSUM chunk, accumulate all runs, relu^2 to hst; finally out-project
per token-tile with t1sq scaling.
"""

import hashlib

import numpy as np
import ml_dtypes

_CACHE = {}

P = 128
D, E, I, DO = 1024, 8, 2048, 1024
DD, II = D // P, I // P                      # 8, 16
NCORES = 8
NTOK = 8192                                  # total tokens (4*2048)
PSUM_BANK = 512


# ---------------------------------------------------------------- host plan

def _greedy_set_order(sets):
    """Order 4-of-8 bitmask sets to maximize adjacent intersections (3 of 4
    shared -> expert runs merge across windows)."""
    rest = list(sets)
    cur = rest.pop(0)
    order = [cur]
    while rest:
        best, bi = -1, 0
        for i, s in enumerate(rest):
            inter = bin(cur & s).count("1")
            if inter > best:
                best, bi = inter, i
        cur = rest.pop(bi)
        order.append(cur)
    return order


def _make_plan(xf, gate_w):
    """Compute routing + the SPMD token layout. Returns dict with:
    Tp, chunks, runs (per expert, chunk-split col ranges), per-core slot->
    token map, sign and t1sq per token."""
    scores = xf.astype(np.float64) @ gate_w.astype(np.float64).T   # [N, E]
    order8 = np.argsort(-scores, axis=1)
    top1 = scores[np.arange(len(xf)), order8[:, 0]].astype(np.float32)
    masks = np.zeros(len(xf), dtype=np.int64)
    for k in range(4):
        masks |= 1 << order8[:, k]

    uniq = np.unique(masks)
    set_order = _greedy_set_order(list(uniq))

    tok_of = {s: np.nonzero(masks == s)[0] for s in set_order}
    widths = []
    for s in set_order:
        n = len(tok_of[s])
        widths.append(n // NCORES + (1 if n % NCORES else 0))
    slots = int(np.sum(widths))
    Tp = ((slots + P - 1) // P) * P

    # psum chunks (<=512 cols each)
    chunks = []
    c0 = 0
    while c0 < Tp:
        cw = min(PSUM_BANK, Tp - c0)
        chunks.append((c0, cw))
        c0 += cw

    # per-core slot -> global token id (-1 dummy)
    slot_tok = np.full((NCORES, Tp), -1, dtype=np.int64)
    off = 0
    for s, w in zip(set_order, widths):
        toks = tok_of[s]
        n = len(toks)
        base, r = n // NCORES, n % NCORES
        pos = 0
        for c in range(NCORES):
            take = base + (1 if c < r else 0)
            slot_tok[c, off:off + take] = toks[pos:pos + take]
            pos += take
        off += w

    # expert runs over slot space: merge adjacent windows, split at chunk
    # boundaries. runs[e] = list of (a, b) absolute col ranges.
    offs = np.concatenate([[0], np.cumsum(widths)])
    runs = []
    for e in range(E):
        act = [(int(offs[i]), int(offs[i + 1]))
               for i, s in enumerate(set_order) if (s >> e) & 1]
        merged = []
        for a, b in act:
            if merged and merged[-1][1] == a:
                merged[-1][1] = b
            else:
                merged.append([a, b])
        split = []
        for a, b in merged:
            while a < b:
                c = min(b, (a // PSUM_BANK + 1) * PSUM_BANK)
                split.append((a, c))
                a = c
        runs.append(split)

    return {
        "Tp": Tp, "chunks": chunks, "runs": runs, "slot_tok": slot_tok,
        "top1": top1,
        "key": hashlib.sha256(
            np.ascontiguousarray(slot_tok).tobytes()
            + repr(runs).encode()).hexdigest(),
    }


# ------------------------------------------------------------- device build

def _split_sync_waits(nc):
    """walrus in this container caps sync waits per instruction (and rejects
    any wait on Drain). Move excess waits onto injected same-engine NOPs
    placed immediately before the instruction - the engine blocks on the
    nops' waits first, so the ordering semantics are identical."""
    from concourse import mybir

    uid = 0
    for bb in nc.m.functions[0].blocks:
        insts = bb.instructions
        new = []
        changed = False
        for inst in insts:
            si = getattr(inst, "sync_info", None)
            waits = list(si.on_wait) if si is not None and si.on_wait else []
            keep = 0 if isinstance(inst, mybir.InstDrain) else 1
            if len(waits) > keep:
                moved, kept = waits[: len(waits) - keep], waits[len(waits) - keep:]
                si.on_wait = kept
                for w in moved:
                    nop = mybir.InstNoOp(
                        name=f"wsplit-{uid}",
                        engine=inst.engine,
                        bass_nofuse=True,
                        sync_info=mybir.SyncInfo(on_wait=[w], on_update=[]),
                    )
                    uid += 1
                    new.append(nop)
                changed = True
            new.append(inst)
        if changed:
            bb.instructions = new
    return nc


def _dedup_ldweights(nc):
    """Legalization inserts one InstLdweights per matmul. Consecutive
    matmuls in the same (expert, dd) share the stationary tile, so reloading
    is pure overhead - drop an LS whose weights AP matches the previous LS
    on the PE stream with only plain matmuls in between. Only sync-free LS
    are dropped (ones carrying waits/updates order other engines)."""
    from concourse import mybir

    def sig(inst):
        ap = inst.ins[0]
        return (getattr(ap, "memref", None), getattr(ap, "offset", None),
                str(getattr(ap, "ap", None)), getattr(ap, "dtype", None),
                inst.perf_mode, inst.is_transpose, inst.tile_position)

    ndrop = 0
    for bb in nc.m.functions[0].blocks:
        last = None
        new = []
        for inst in bb.instructions:
            if isinstance(inst, mybir.InstLdweights):
                si = getattr(inst, "sync_info", None)
                clean = si is None or (not si.on_wait and not si.on_update)
                s = sig(inst)
                if clean and last is not None and s == last:
                    ndrop += 1
                    continue
                last = s
            elif isinstance(inst, mybir.InstMatmult):
                if inst.is_transpose:
                    last = None
            elif isinstance(inst, mybir.InstNoOp):
                pass
            elif getattr(inst, "engine", None) == mybir.EngineType.PE:
                last = None
            new.append(inst)
        bb.instructions = new
    return ndrop


def _build_nc(reps=1, split_waits=True, plan=None):
    import contextlib

    import concourse.bass as bass
    import concourse.mybir as mybir
    import concourse.tile as tile

    if plan is None:
        plan = _CACHE["plan"]
    Tp, chunks, runs = plan["Tp"], plan["chunks"], plan["runs"]
    TT = Tp // P

    f32 = mybir.dt.float32
    bf16 = mybir.dt.bfloat16
    Alu = mybir.AluOpType
    Act = mybir.ActivationFunctionType

    nc = bass.Bass("TRN2", target_bir_lowering=False, debug=False)
    xT_d = nc.dram_tensor("xT", [P, DD, Tp], bf16, kind="ExternalInput")
    t1_d = nc.dram_tensor("t1sq", [P, TT], f32, kind="ExternalInput")
    # expert weights pre-tiled on host: [ii, d_inner, e, dd, i_inner] so one
    # i-tile's worth of all experts is a single fully-contiguous 2MB DMA
    ewt_d = nc.dram_tensor("ewt", [II, P, E, DD, P], bf16, kind="ExternalInput")
    owt_d = nc.dram_tensor("owt", [I, DO], bf16, kind="ExternalInput")
    out_d = nc.dram_tensor("out", [Tp, DO], f32, kind="ExternalOutput")

    outr = out_d.rearrange("(tt p) d -> p tt d", p=P)
    owr = owt_d.rearrange("(ii p) d -> p ii d", p=P)

    with tile.TileContext(nc) as tc:
        with (
            tc.tile_pool(name="const", bufs=1) as constp,
            tc.tile_pool(name="ewp", bufs=3) as ewp,
            tc.tile_pool(name="hstp", bufs=1) as hstp,
            tc.tile_pool(name="rp", bufs=2) as rp,
            tc.tile_pool(name="obp", bufs=2) as obp,
            tc.tile_pool(name="ps_hs", bufs=2, space="PSUM") as psh,
            tc.tile_pool(name="ps_out", bufs=2, space="PSUM") as pso,
        ):
            xT16 = constp.tile([P, DD, Tp], bf16)
            t1sq = constp.tile([P, TT], f32)
            ow_sb = constp.tile([P, II, DO], bf16)

            loop_cm = (
                tc.For_i(
                    0, reps, 1,
                    hint_engines=(
                        mybir.EngineType.PE, mybir.EngineType.DVE,
                        mybir.EngineType.Activation, mybir.EngineType.SP,
                        mybir.EngineType.Pool,
                    ),
                )
                if reps > 1 else contextlib.nullcontext()
            )
            with loop_cm:
                hst = hstp.tile([P, II, Tp], bf16, tag="hst")
                for ii in range(II):
                    ew = ewp.tile([P, E, DD, P], bf16, tag="ew")
                    # per-expert sub-loads: the first matmuls are gated on
                    # 256KB (expert 0) instead of the full 2MB tile
                    for e in range(E):
                        nc.sync.dma_start(ew[:, e, :, :], ewt_d[ii, :, e, :, :])
                        if ii == 0:
                            # interleave x dd-slices with the first tile's
                            # experts so (e<=k, dd<=k) matmuls start early
                            nc.sync.dma_start(xT16[:, e, :], xT_d[:, e, :])
                    if ii == 0:
                        nc.sync.dma_start(t1sq[:], t1_d[:, :])
                    if ii == 1:
                        # out_w load deferred so the first ew tiles are not
                        # queued behind this 4MB transfer
                        nc.sync.dma_start(ow_sb[:], owr[:, :, :])
                    # chunk innermost: one (e, dd) stationary covers every
                    # chunk's runs (the dedup pass keeps a single Ldweights
                    # per (e, dd)); DVE pre-zeroes each bank so runs
                    # accumulate without start-flag bookkeeping
                    hps = {}
                    left = {}
                    for (c0, cw) in chunks:
                        hp = psh.tile([P, cw], f32, tag=f"hp{c0}")
                        nc.vector.memset(hp[:, :], 0.0)
                        hps[c0] = hp
                        left[c0] = DD * sum(
                            1 for e in range(E) for (a, b) in runs[e]
                            if c0 <= a < c0 + cw
                        )
                    for e in range(E):
                        for dd in range(DD):
                            for (c0, cw) in chunks:
                                for (a, b) in runs[e]:
                                    if not (c0 <= a < c0 + cw):
                                        continue
                                    left[c0] -= 1
                                    nc.tensor.matmul(
                                        hps[c0][:, a - c0:b - c0],
                                        ew[:, e, dd, :],
                                        xT16[:, dd, a:b],
                                        start=False,
                                        stop=(left[c0] == 0),
                                        skip_group_check=True,
                                    )
                    for (c0, cw) in chunks:
                        rt = rp.tile([P, cw], f32, tag=f"rt{c0}")
                        nc.scalar.activation(rt[:], hps[c0][:], Act.Relu)
                        nc.scalar.activation(
                            hst[:, ii, c0:c0 + cw], rt[:], Act.Square
                        )

                for tl in range(TT):
                    # dc inner so both halves share each (tl, ii) stationary
                    # (the Ldweights dedup pass then drops every second load)
                    ops0 = pso.tile([P, 512], f32, tag="ops0")
                    ops1 = pso.tile([P, 512], f32, tag="ops1")
                    ops2 = [ops0, ops1]
                    for ii in range(II):
                        for dc in range(2):
                            nc.tensor.matmul(
                                ops2[dc][:], hst[:, ii, tl * P:(tl + 1) * P],
                                ow_sb[:, ii, dc * 512:(dc + 1) * 512],
                                start=(ii == 0), stop=(ii == II - 1),
                            )
                    for dc in range(2):
                        ob = obp.tile([P, 512], f32, tag="ob")
                        nc.vector.tensor_scalar(
                            ob[:], ops2[dc][:], t1sq[:, tl:tl + 1], None,
                            Alu.mult
                        )
                        nc.sync.dma_start(
                            outr[:, tl, dc * 512:(dc + 1) * 512], ob[:]
                        )
    _dedup_ldweights(nc)
    if split_waits:
        _split_sync_waits(nc)
    return nc


# ------------------------------------------------------------------- driver

def _make_in_maps(inputs):
    x = inputs["x"]
    top_k = int(inputs["top_k"])
    assert top_k == 4, f"kernel hardcodes top_k=4, got {top_k}"
    gate_w = np.asarray(inputs["gate_w"], np.float32)
    expert_w, out_w = inputs["expert_w"], inputs["out_w"]
    B, S, Dm = x.shape
    assert (Dm, gate_w.shape[0], expert_w.shape[1], out_w.shape[0]) == (D, E, I, DO)
    xf = np.ascontiguousarray(np.asarray(x, dtype=np.float32).reshape(-1, Dm))
    assert xf.shape[0] == NTOK

    plan = _make_plan(xf, gate_w)
    _CACHE["plan"] = plan
    Tp, slot_tok, top1 = plan["Tp"], plan["slot_tok"], plan["top1"]
    TT = Tp // P

    bf = ml_dtypes.bfloat16
    ewt = np.ascontiguousarray(
        np.asarray(expert_w, np.float32)
        .reshape(E, II, P, DD, P)
        .transpose(1, 4, 0, 3, 2)
    ).astype(bf)
    owt = np.ascontiguousarray(np.asarray(out_w, np.float32).T).astype(bf)

    sign = np.where(top1 >= 0, np.float32(1.0), np.float32(-1.0))
    t1sq_full = (top1 * top1).astype(np.float32)

    in_maps = []
    for c in range(NCORES):
        st = slot_tok[c]
        real = st >= 0
        xs = np.zeros((Tp, D), dtype=np.float32)
        xs[real] = xf[st[real]] * sign[st[real], None]
        # xT16[p, dd, t] = xs[t, dd*128 + p]
        xT16 = np.ascontiguousarray(
            xs.reshape(Tp, DD, P).transpose(2, 1, 0)).astype(bf)
        t1 = np.zeros(Tp, dtype=np.float32)
        t1[real] = t1sq_full[st[real]]
        t1 = np.ascontiguousarray(t1.reshape(TT, P).T)   # [P, TT]
        in_maps.append({"xT": xT16, "t1sq": t1, "ewt": ewt, "owt": owt})
    return in_maps


def _get_nc():
    plan = _CACHE["plan"]
    key = plan["key"]
    if _CACHE.get("nc_key") != key:
        _CACHE["nc"] = _build_nc(plan=plan)
        _CACHE["nc_key"] = key
    return _CACHE["nc"]


def kernel(x, gate_w, expert_w, out_w, top_k):
    from concourse.bass_utils import run_bass_kernel_spmd

    in_maps = _make_in_maps(dict(
        x=x, gate_w=gate_w, expert_w=expert_w, out_w=out_w, top_k=top_k
    ))
    plan = _CACHE["plan"]
    nc = _get_nc()
    res = run_bass_kernel_spmd(nc, in_maps, list(range(NCORES)))

    B, S, Dm = x.shape
    out = np.zeros((NTOK, DO), dtype=np.float32)
    slot_tok = plan["slot_tok"]
    for c in range(NCORES):
        st = slot_tok[c]
        real = st >= 0
        out[st[real]] = res.results[c]["out"][real]
    return out.reshape(B, S, Dm).astype(np.float32)
